# revision 42
# baseline (speedup 1.0000x reference)
"""Trainium2 Bass kernel for nn_MoELayer (moe_routing).

Reference computation (B=8192 tokens, d=1024, E=8 experts, top-k=2):
    gating  = softmax(x @ gate_w + gate_b)                    # [B, E]
    mask    = top-2 one-hot scatter of gating                 # [B, E]
    blockm  = mask.reshape(B//d, d, E).max(axis=1)            # per 1024-row block
    out     = sum_e gating[:, e] * blockm[block(b), e] * (x @ W[:, e*d:(e+1)*d])

Key structural facts exploited here:
  * The combine uses the FULL softmax weights over all experts; the top-2
    mask enters only through the per-1024-row-block max. So the compute is
    dense: out = sum_e (gating*blockmask) .* (x @ W_e).
  * Both the gating and the block mask for a 1024-row block depend only on
    that block's rows.

Sharding: data-parallel over the 8 row blocks of 1024 tokens (one per
NeuronCore). Each core computes its whole output block locally; there is
no cross-core communication. W is streamed (replicated) to every core.

Optimizations over the first working version (256.2us -> ~248.5us):
  * x^T, W and gate_w are converted to bf16 ON THE HOST and DMA'd directly
    into the tiles the PE reads (halves input DMA bytes, removes every DVE
    f32->bf16 conversion from the startup critical path). The output also
    leaves as bf16 (host upcasts); ~0.2% extra rounding vs the 2e-2 gate.
  * Expert 0 is computed UNSCALED in two k-passes (k0-1, then k2-7) sized
    to the ~230 GB/s effective per-core HBM rate so the PE starts as soon
    as the first two (x, W0) k-tile pairs land.
  * gate_w/gate_b DMAs ride the ACT-engine DGE ring — the Sync ring's
    serialized W-tile triggers (~0.6-1us each) would delay them by ~5us.
  * Eviction is one fused DVE op (scalar_tensor_tensor: acc = ps*g + acc)
    instead of ACT mul + DVE add — halves vector-engine work (less
    activity throttling) and shortens the drain.
  * Expert 7 stores each half-tile the moment it is final, alternating
    DGE rings, with the very last tile stored in quarter-width chunks —
    the output-DMA trigger chain spreads across expert 7 instead of
    serializing ~9us after the last matmul.
  * Gating logits ride INSIDE expert 0's matmul stream: one 8-column
    matmul per (m, k) reusing the expert matmuls' stationary x-tile
    (~15ns each, vs ~4us for a separate [E, tokens] logits pass at 8/128
    PE utilization plus a transpose pass). Every contribution is its own
    single-shot psum group at a distinct offset of one dedicated bank —
    multi-instruction accumulation groups interleaved within a bank do
    NOT survive scheduling — and a per-m strided DVE reduce sums over k
    as soon as that m's contributions complete. gate_b enters exactly
    via exp(l+b) = exp(l)*exp(b) with a broadcast exp(gate_b) row.
  * A PE warm-up sized to the DMA-bound startup window keeps the HAM
    clock-gate at 2.4 GHz until the first expert's tiles land.

Measured residual (unavoidable here): ~8.7us NEFF teardown (walrus's
per-engine semaphore-zero chains + final barrier), ~2us framework
preamble inside the measured window, ~2.5us DMA-rate-bound startup, and
~4us activity-throttle (HW power management) at sustained full PE rate.
"""

import numpy as np

P = 128          # partitions
D = 1024         # d_model
E = 8            # experts
TOK = 1024       # tokens per core (row block)
KT = D // P      # contraction tiles
KH = 2           # expert-0 pass A k-depth (matched to DMA arrival order)
MT = TOK // P    # token tiles
NH = 512         # psum half-width (one fp32 bank)
QW = 256         # quarter width for the fine-grained final eviction
N_CORES = 8
WARMUP_MMS = 6   # PE warm-up matmuls issued while the first DMAs land
                 # (the startup is DMA-bound until ~13us; warm-ups bridge
                 # the gap and keep the HAM clock-gate from dropping)


def _build_nc():
    import concourse.bacc as bacc
    import concourse.mybir as mybir
    import concourse.tile as tile

    f32 = mybir.dt.float32
    bf16 = mybir.dt.bfloat16
    AX = mybir.AxisListType
    OP = mybir.AluOpType
    AF = mybir.ActivationFunctionType

    # Bacc (not raw Bass): its compile() pass splits excess per-instruction
    # semaphore waits into standalone event-semaphore instructions and moves
    # matmul waits onto LDWEIGHTS — required for walrus codegen's per-
    # instruction sync-wait limits.
    nc = bacc.Bacc(None, target_bir_lowering=False, debug=False)
    xT_d = nc.dram_tensor("xT", [D, TOK], bf16, kind="ExternalInput")
    w_d = nc.dram_tensor("w", [D, E * D], bf16, kind="ExternalInput")
    gw_d = nc.dram_tensor("gate_w", [D, E], bf16, kind="ExternalInput")
    gb_d = nc.dram_tensor("gate_b", [1, E], f32, kind="ExternalInput")
    # output leaves as bf16 (host upcasts): halves the store DMA bytes; the
    # rounding it adds (~0.2% elementwise) is well inside the 2e-2 gate
    out_d = nc.dram_tensor("out", [TOK, D], bf16, kind="ExternalOutput")

    xT_r = xT_d.rearrange("(k p) t -> k p t", p=P)
    w_r = w_d.rearrange("(k p) (e f) -> k p e f", p=P, f=D)
    gw_r = gw_d.rearrange("(k p) e -> p k e", p=P)
    out_r = out_d.rearrange("(m p) f -> m p f", p=P)

    with tile.TileContext(nc) as tc:
        with (
            tc.tile_pool(name="persist", bufs=1) as persist,
            tc.tile_pool(name="gstat", bufs=2) as p_gs,
            tc.tile_pool(name="wb", bufs=2 * KT) as p_wb,
            tc.tile_pool(name="tmp", bufs=6) as p_tmp,
            tc.tile_pool(name="ps_gate", bufs=1, space="PSUM") as ps_gate,
            tc.tile_pool(name="ps_cnt", bufs=1, space="PSUM") as ps_cnt,
            tc.tile_pool(name="ps_mm", bufs=6, space="PSUM") as ps_mm,
        ):
            # -- front matter: everything with no DMA dependency goes first so
            # the PE warm-up and the ACT exp-table load start at t~0.
            wu_lhs = persist.tile([P, P], bf16, tag="wu_lhs")
            nc.vector.memset(wu_lhs[:], 0.0)
            wu_rhs = persist.tile([P, NH], bf16, tag="wu_rhs")
            nc.vector.memset(wu_rhs[:], 0.0)
            ones_col = persist.tile([P, 1], bf16, tag="ones_col")
            nc.vector.memset(ones_col[:], 1.0)
            ones_row = persist.tile([1, P], f32, tag="ones_row")
            nc.vector.memset(ones_row[:], 1.0)
            exp_in = persist.tile([P, 1], f32, tag="exp_in")
            nc.vector.memset(exp_in[:], 1.0)
            ones_row_bf = persist.tile([1, P], bf16, tag="ones_row_bf")
            nc.vector.memset(ones_row_bf[:], 1.0)
            # Preload the exp activation-table set (~2.7us) during DMA wait.
            exp_dummy = persist.tile([1, 1], f32, tag="exp_dummy")
            nc.scalar.activation(exp_dummy[:], exp_in[:1, :], AF.Exp)

            # PE warm-up while the first input DMAs are in flight: keeps HAM
            # busy so the real matmuls run at 2.4 GHz.
            wu_ps = ps_cnt.tile([P, NH], f32, tag="cnt")
            for i in range(WARMUP_MMS):
                nc.tensor.matmul(
                    wu_ps[:], wu_lhs[:], wu_rhs[:],
                    start=(i == 0), stop=(i == WARMUP_MMS - 1),
                )

            # -- loads: x^T k-tiles interleaved with expert-0 weight k-tiles
            # so expert 0's first matmuls can start as early as possible.
            # All tiles are bf16 straight from DRAM — the PE reads them with
            # no conversion step in between.
            xtb = []
            wbf0 = []
            for k in range(KT):
                xb = persist.tile([P, TOK], bf16, tag=f"xtb{k}")
                nc.sync.dma_start(xb[:], xT_r[k])
                xtb.append(xb)
                wb = p_wb.tile([P, D], bf16, tag="wb", name=f"wb0_{k}")
                nc.sync.dma_start(wb[:], w_r[k, :, 0, :])
                wbf0.append(wb)

            # gate weights/bias ride the ACT-engine DGE ring: the Sync ring
            # has a deep backlog of W-tile triggers (each ~0.6-1us), which
            # would otherwise delay the gating-logit matmuls by several us.
            gb_in = persist.tile([1, E], f32, tag="gb_in")
            nc.scalar.dma_start(gb_in[:], gb_d[:])
            # gate_b enters the softmax multiplicatively: exp(l+b) =
            # exp(l)*exp(b). ebg_row is broadcast to all partitions later.
            ebg_row = persist.tile([1, E], f32, tag="ebg_row")
            nc.scalar.activation(ebg_row[:], gb_in[:], AF.Exp)
            ebg_bf = persist.tile([1, E], bf16, tag="ebg_bf")
            nc.vector.tensor_copy(ebg_bf[:], ebg_row[:])
            bmb = persist.tile([P, E], f32, tag="bmb")
            gw_bf = persist.tile([P, KT, E], bf16, tag="gw_bf")
            nc.scalar.dma_start(gw_bf[:], gw_r[:])

            # acc: the fp32 output accumulator. acc0: expert 0 computed
            # UNSCALED (kept in bf16; it is a pre-gating matmul result, so
            # bf16 storage costs the same precision as the bf16 matmul
            # inputs already do). Its gating scale is folded in during
            # experts 2-3, which removes the gating computation from the
            # startup critical path.
            acc = []
            acc0 = []
            acc1 = []
            obf = []
            for m in range(MT):
                acc.append(persist.tile([P, D], f32, tag=f"acc{m}",
                                        name=f"acc{m}"))
                acc0.append(persist.tile([P, D], bf16, tag=f"acc0{m}",
                                         name=f"acc0{m}"))
                acc1.append(persist.tile([P, D], bf16, tag=f"acc1{m}",
                                         name=f"acc1{m}"))
                obf.append(persist.tile([P, D], bf16, tag=f"obf{m}",
                                        name=f"obf{m}"))

            # Dependency-free filler matmul: keeps the PE's HAM activity
            # monitor busy during arrival-gated stretches so the clock stays
            # at 2.4 GHz. Uses the bmb_ps bank (its real use comes later).
            fill_ps = ps_gate.tile([P, NH], f32, tag="bmb_ps", bufs=1)

            def pe_filler(n=1):
                for _ in range(n):
                    nc.tensor.matmul(fill_ps[:, :NH], wu_lhs[:], wu_rhs[:],
                                     start=True, stop=True)

            # Gating logits ride INSIDE expert 0's matmul stream: each (m,k)
            # step appends one 8-column matmul that reuses the expert
            # matmuls' stationary x-tile (~15ns each vs ~4us for a separate
            # [E, tokens] logits pass at 8/128 PE utilization). Logits come
            # out directly in [tokens, E] layout, so no transpose pass is
            # needed either. Every contribution is its OWN single-shot
            # (start+stop) psum group at a distinct offset — multi-
            # instruction accumulation groups interleaved within a bank do
            # not survive scheduling — and DVE reduces over k at softmax
            # time. 8k x 8m x 8E fp32 = exactly one psum bank.
            lg_full = ps_cnt.tile([P, KT * MT * E], f32, tag="cnt")

            # -- expert 0, pass A (k = 0..1), unscaled -> acc0; only the
            # first two (x,w) k-tile pairs need to have landed for it to run
            for m in range(MT):
                ps0 = ps_mm.tile([P, NH], f32, tag="psmm")
                ps1 = ps_mm.tile([P, NH], f32, tag="psmm")
                for k in range(KH):
                    lhs = xtb[k][:, m * P:(m + 1) * P]
                    nc.tensor.matmul(ps0[:], lhs, wbf0[k][:, 0:NH],
                                     start=(k == 0), stop=(k == KH - 1))
                    nc.tensor.matmul(ps1[:], lhs, wbf0[k][:, NH:D],
                                     start=(k == 0), stop=(k == KH - 1))
                    nc.tensor.matmul(
                        lg_full[:, k * MT * E + m * E:k * MT * E + (m + 1) * E],
                        lhs, gw_bf[:, k, :], start=True, stop=True)
                    if m == 0:
                        pe_filler(2)
                nc.scalar.copy(acc0[m][:, 0:NH], ps0[:])
                nc.scalar.copy(acc0[m][:, NH:D], ps1[:])

            def load_w(e):
                tiles = []
                for k in range(KT):
                    wb = p_wb.tile([P, D], bf16, tag="wb", name=f"wb{e}_{k}")
                    nc.sync.dma_start(wb[:], w_r[k, :, e, :])
                    tiles.append(wb)
                return tiles

            # Prefetch expert 1's weights behind expert 0's stream.
            wbf_cur = load_w(1)

            # -- expert 0, pass B (k = 2..7), accumulate into acc0 on DVE.
            # Each m's logit contributions complete here, so its k-reduction
            # is emitted right away — the softmax chains then spread across
            # experts 0-1 instead of bunching up and blocking the PE queue
            # behind the mask-count matmuls.
            lg_sum = persist.tile([P, MT * E], f32, tag="lg_sum")
            for m in range(MT):
                ps0 = ps_mm.tile([P, NH], f32, tag="psmm")
                ps1 = ps_mm.tile([P, NH], f32, tag="psmm")
                for k in range(KH, KT):
                    lhs = xtb[k][:, m * P:(m + 1) * P]
                    nc.tensor.matmul(ps0[:], lhs, wbf0[k][:, 0:NH],
                                     start=(k == KH), stop=(k == KT - 1))
                    nc.tensor.matmul(ps1[:], lhs, wbf0[k][:, NH:D],
                                     start=(k == KH), stop=(k == KT - 1))
                    nc.tensor.matmul(
                        lg_full[:, k * MT * E + m * E:k * MT * E + (m + 1) * E],
                        lhs, gw_bf[:, k, :], start=True, stop=True)
                nc.vector.tensor_reduce(
                    lg_sum[:, m * E:(m + 1) * E],
                    lg_full[:].rearrange("p (k m e) -> p m e k",
                                         k=KT, e=E)[:, m],
                    axis=AX.X, op=OP.add)
                nc.vector.tensor_tensor(acc0[m][:, 0:NH], acc0[m][:, 0:NH],
                                        ps0[:], op=OP.add)
                nc.vector.tensor_tensor(acc0[m][:, NH:D], acc0[m][:, NH:D],
                                        ps1[:], op=OP.add)

            # Gating part 2: per-token-tile transpose of the logits via a
            # K=8 matmul against an 8x8 identity, then softmax + top-2 mask.
            # Masks are packed into one [P, MT*E] tile so the block-mask
            # count is a single matmul.
            mask_all = persist.tile([P, MT * E], bf16, tag="mask_all")
            gfin = []
            gsc = [persist.tile([P, E], f32, tag=f"gsc{m}", name=f"gsc{m}")
                   for m in range(MT)]
            # broadcast exp(gate_b) to all partitions via a K=1 matmul
            ebg_ps = ps_gate.tile([P, E], f32, tag="bmb_ps", bufs=1)
            nc.tensor.matmul(ebg_ps[:], ones_row_bf[:], ebg_bf[:],
                             start=True, stop=True)
            ebg_b = persist.tile([P, E], f32, tag="ebg_b")
            nc.vector.tensor_copy(ebg_b[:], ebg_ps[:])
            for m in range(MT):
                # softmax over the 8 experts (free dim). |logit| is O(1), so
                # no max-subtraction is needed for fp32 exp.
                ex = p_gs.tile([P, E], f32, tag="ex")
                nc.scalar.activation(ex[:], lg_sum[:, m * E:(m + 1) * E],
                                     AF.Exp)
                nc.vector.tensor_tensor(ex[:], ex[:], ebg_b[:], op=OP.mult)
                ssum = p_gs.tile([P, 1], f32, tag="ssum")
                nc.vector.reduce_sum(ssum[:], ex[:], axis=AX.X)
                rcp = p_gs.tile([P, 1], f32, tag="rcp")
                nc.vector.reciprocal(rcp[:], ssum[:])
                # top-2 mask: v >= (max of values with the argmax removed)
                m1 = p_gs.tile([P, 1], f32, tag="m1")
                nc.vector.reduce_max(m1[:], ex[:], axis=AX.X)
                eqb = p_gs.tile([P, E], f32, tag="eqb")
                nc.vector.tensor_scalar(
                    eqb[:], ex[:], m1[:], -1e30, op0=OP.is_ge, op1=OP.mult
                )
                g2 = p_gs.tile([P, E], f32, tag="g2")
                nc.vector.tensor_tensor(g2[:], ex[:], eqb[:], op=OP.add)
                m2 = p_gs.tile([P, 1], f32, tag="m2")
                nc.vector.reduce_max(m2[:], g2[:], axis=AX.X)
                nc.vector.tensor_scalar(mask_all[:, m * E:(m + 1) * E],
                                        ex[:], m2[:], None, op0=OP.is_ge)
                gt = p_gs.tile([P, E], f32, tag=f"gt{m}", bufs=1)
                nc.vector.tensor_scalar_mul(gt[:], ex[:], rcp[:])
                gfin.append(gt)

            # block-mask: one ones^T @ mask matmul over all m tiles at once,
            # then a strided reduce over the m axis.
            cnt_ps = ps_cnt.tile([1, MT * E], f32, tag="cnt")
            nc.tensor.matmul(cnt_ps[:], ones_col[:], mask_all[:],
                             start=True, stop=True)
            cnt_sb = p_gs.tile([1, MT * E], f32, tag="cnt_sb")
            nc.vector.tensor_copy(cnt_sb[:], cnt_ps[:])
            cnt_e = p_gs.tile([1, E], f32, tag="cnt_e")
            nc.vector.tensor_reduce(
                cnt_e[:], cnt_sb[:].rearrange("p (m e) -> p e m", e=E),
                axis=AX.X, op=OP.add,
            )
            bm01 = p_gs.tile([1, E], bf16, tag="bm01")
            nc.vector.tensor_scalar(bm01[:], cnt_e[:], 0.5, None, op0=OP.is_ge)
            # broadcast [1,E] -> [P,E] via K=1 matmul with a ones row
            bmb_ps = ps_gate.tile([P, E], f32, tag="bmb_ps", bufs=1)
            nc.tensor.matmul(bmb_ps[:], ones_row_bf[:], bm01[:],
                             start=True, stop=True)
            nc.vector.tensor_copy(bmb[:], bmb_ps[:])
            for m in range(MT):
                nc.vector.tensor_tensor(gsc[m][:], gfin[m][:], bmb[:],
                                        op=OP.mult)

            # -- experts 1..7: acc (+)= g_e * (x @ W_e); expert 0's scaled
            # contribution g0 * acc0 is merged in during experts 2 and 3.
            for e in range(1, E):
                wbf = wbf_cur
                if e + 1 < E:
                    wbf_cur = load_w(e + 1)
                for m in range(MT):
                    ps0 = ps_mm.tile([P, NH], f32, tag="psmm")
                    ps1 = ps_mm.tile([P, NH], f32, tag="psmm")
                    for k in range(KT):
                        lhs = xtb[k][:, m * P:(m + 1) * P]
                        nc.tensor.matmul(ps0[:], lhs, wbf[k][:, 0:NH],
                                         start=(k == 0), stop=(k == KT - 1))
                        nc.tensor.matmul(ps1[:], lhs, wbf[k][:, NH:D],
                                         start=(k == 0), stop=(k == KT - 1))
                    for h, ps in ((0, ps0), (1, ps1)):
                        osl = acc[m][:, h * NH:(h + 1) * NH]
                        if e == E - 1 and m == MT - 1:
                            # final tile: quarter-width fused-DVE/DMA
                            # pipeline (split across both DGE rings) so the
                            # post-matmul drain is as short as possible
                            for q in range(2):
                                off = h * NH + q * QW
                                oq = obf[m][:, off:off + QW]
                                nc.vector.scalar_tensor_tensor(
                                    oq, ps[:, q * QW:(q + 1) * QW],
                                    gsc[m][:, e:e + 1],
                                    acc[m][:, off:off + QW],
                                    op0=OP.mult, op1=OP.add)
                                eng = nc.scalar if q == 0 else nc.sync
                                eng.dma_start(out_r[m][:, off:off + QW], oq)
                            continue
                        if e == 1:
                            # expert 1 is also computed unscaled (no gating
                            # dependency); merged with its gate later.
                            nc.scalar.copy(acc1[m][:, h * NH:(h + 1) * NH],
                                           ps[:])
                        elif e == 2:
                            nc.scalar.mul(osl, ps[:], gsc[m][:, e:e + 1])
                        elif e < E - 1:
                            # single fused DVE op: acc = ps*g + acc (replaces
                            # an ACT mul + DVE add; halves vector-engine work)
                            nc.vector.scalar_tensor_tensor(
                                osl, ps[:], gsc[m][:, e:e + 1], osl,
                                op0=OP.mult, op1=OP.add)
                        else:
                            # last expert: the fused DVE op writes the bf16
                            # output tile directly, then stores as soon as
                            # the half is final so the DMA-trigger chains
                            # (~0.6us per DMA) spread across expert 7 instead
                            # of serializing after the last matmul
                            ob = obf[m][:, h * NH:(h + 1) * NH]
                            nc.vector.scalar_tensor_tensor(
                                ob, ps[:], gsc[m][:, e:e + 1], osl,
                                op0=OP.mult, op1=OP.add)
                            eng = nc.scalar if h == 0 else nc.sync
                            eng.dma_start(out_r[m][:, h * NH:(h + 1) * NH], ob)
                    if e in (3, 4, 5, 6):
                        # merge the unscaled experts: acc += g0*acc0 (e 3-4)
                        # and acc += g1*acc1 (e 5-6), half the m tiles each
                        merge_e = 0 if e in (3, 4) else 1
                        if (e % 2 == 1) == (m < MT // 2):
                            a_un = acc0 if merge_e == 0 else acc1
                            gcol = gsc[m][:, merge_e:merge_e + 1]
                            for h in range(2):
                                osl = acc[m][:, h * NH:(h + 1) * NH]
                                asl = a_un[m][:, h * NH:(h + 1) * NH]
                                nc.vector.scalar_tensor_tensor(
                                    osl, asl, gcol, osl,
                                    op0=OP.mult, op1=OP.add)

            # (output stores are issued inside expert 7's loop above)

    nc.compile()
    return nc


def _ensure_ntff_hook_module():
    """Defensive: some environments lack ``antenv.axon_hooks``; if a caller
    sets BASS_TRACE=1, run_bass_kernel_spmd imports it unconditionally and
    would crash. Provide a working shim (wired to the axon profiler if the
    library is present, else a no-hook stub)."""
    import sys
    import types

    try:
        import antenv.axon_hooks  # noqa: F401
        return
    except ImportError:
        pass
    try:
        import antenv  # noqa: F401
    except ImportError:
        return
    m = types.ModuleType("antenv.axon_hooks")
    exec(
        "_hook = None\n"
        "def set_axon_ntff_profile_hook(h):\n"
        "    global _hook\n"
        "    _hook = h\n"
        "def get_axon_ntff_profile_hook():\n"
        "    return _hook\n",
        m.__dict__,
    )
    sys.modules["antenv.axon_hooks"] = m
    try:
        from trn_agent_boot.trn_boot import _ntff_profile_via_ctypes

        m.set_axon_ntff_profile_hook(
            _ntff_profile_via_ctypes("/opt/axon/libaxon_pjrt.so")
        )
    except Exception:
        pass


_ensure_ntff_hook_module()

_CACHE = {}
LAST_RESULTS = None  # BassKernelResults of the most recent run (for test.py)


def _get_nc():
    if "nc" not in _CACHE:
        _CACHE["nc"] = _build_nc()
    return _CACHE["nc"]


def kernel(x, W, gate_w, gate_b):
    global LAST_RESULTS
    import ml_dtypes
    from concourse.bass_utils import run_bass_kernel_spmd

    bf16 = ml_dtypes.bfloat16
    x = np.asarray(x, dtype=np.float32)
    Wb = np.ascontiguousarray(np.asarray(W, dtype=np.float32).astype(bf16))
    gwb = np.ascontiguousarray(
        np.asarray(gate_w, dtype=np.float32).astype(bf16))
    gb = np.ascontiguousarray(np.asarray(gate_b, dtype=np.float32).reshape(1, E))

    in_maps = []
    for c in range(N_CORES):
        xT = np.ascontiguousarray(x[c * TOK:(c + 1) * TOK].T.astype(bf16))
        in_maps.append({"xT": xT, "w": Wb, "gate_w": gwb, "gate_b": gb})

    res = run_bass_kernel_spmd(_get_nc(), in_maps, core_ids=list(range(N_CORES)))
    LAST_RESULTS = res
    return np.concatenate(
        [np.asarray(r["out"]).astype(np.float32) for r in res.results], axis=0)
